# revision 1
# baseline (speedup 1.0000x reference)
"""ConstraintLoss (segment_reduce) kernel for 8 Trainium2 NeuronCores.

Strategy:
  - Host: sort the nnz entries by constr_idx (radix argsort), lay them into a
    fixed-32-slot-per-constraint padded layout (constraints with >32 nnz spill
    into per-core overflow rows), and shard by constraint range: core k owns
    constraints [k*125000, (k+1)*125000).
  - Device (SPMD, one Bass program on 8 cores): stream the slotted
    pred/coeff arrays, sigmoid (ACT) * coeff (DVE), reduce each group of 32
    slots to ax[c] (DVE tensor_reduce), apply overflow-row sums with an
    indirect-DMA accumulate, compute sense-masked violations, and reduce to a
    per-core partial sum (DVE reduce + TensorE partition reduce).
  - Host: sum the 8 partials / n_constrs.
"""
import sys

if "/opt/trn_rl_repo" not in sys.path:
    sys.path.insert(0, "/opt/trn_rl_repo")

from contextlib import ExitStack

import numpy as np

import concourse.bass as bass
import concourse.tile as tile
from concourse import bacc, mybir
from concourse.bass_utils import run_bass_kernel_spmd

P = 128
K = 32                       # slots per constraint in the main structure
N_CORES = 8
N_VARS = 2_000_000
N_CONSTRS = 1_000_000
NNZ = 20_000_000
CPC = N_CONSTRS // N_CORES   # constraints per core
GPP = (CPC + P - 1) // P     # constraint groups per partition (977)
CPC_PAD = P * GPP            # padded constraints per core (125056)
F32 = mybir.dt.float32
AF = mybir.ActivationFunctionType


def _prep(pred, constr_idx, var_idx, coeff, constr_rhs, constr_sense):
    """Sort by constraint, build padded slot arrays + overflow rows."""
    E = constr_idx.shape[0]
    c = np.asarray(constr_idx)
    order = np.argsort(c, kind="stable")
    sc = c[order]
    counts = np.bincount(sc, minlength=N_CONSTRS)
    starts = np.concatenate([[0], np.cumsum(counts)[:-1]]).astype(np.int64)
    rank = np.arange(E, dtype=np.int64) - starts[sc]
    sv = np.asarray(var_idx)[order]
    scf = np.asarray(coeff)[order]

    core = sc // CPC
    lc = sc - core * CPC

    is_main = rank < K
    m_core = core[is_main]
    m_slot = lc[is_main] * K + rank[is_main]

    is_ov = ~is_main
    ov_core = core[is_ov]
    ov_c = sc[is_ov]
    ov_r = rank[is_ov] - K
    row_within = ov_r // K
    col = ov_r % K
    pair_key = ov_c * 64 + row_within            # supports n_c up to 64*32+32
    new_pair = np.diff(pair_key, prepend=np.int64(-1)) != 0
    row_gid = np.cumsum(new_pair) - 1
    core_of_row = ov_core[new_pair]
    rows_per_core = np.bincount(core_of_row, minlength=N_CORES)
    row_base = np.concatenate([[0], np.cumsum(rows_per_core)[:-1]])
    row_lid = row_gid - row_base[ov_core]
    OVR = max(P, int(np.ceil(max(int(rows_per_core.max() or 0), 1) / P)) * P)

    ov_dest = np.full((N_CORES, OVR), CPC_PAD, dtype=np.int32)   # default: sink
    ov_dest[core_of_row, row_lid[new_pair]] = lc[is_ov][new_pair].astype(np.int32)

    ps = np.zeros((N_CORES, CPC_PAD * K), dtype=np.float32)
    cs = np.zeros((N_CORES, CPC_PAD * K), dtype=np.float32)
    ps[m_core, m_slot] = pred[sv[is_main]]
    cs[m_core, m_slot] = scf[is_main]
    ovp = np.zeros((N_CORES, OVR * K), dtype=np.float32)
    ovc = np.zeros((N_CORES, OVR * K), dtype=np.float32)
    ov_slot = row_lid * K + col
    ovp[ov_core, ov_slot] = pred[sv[is_ov]]
    ovc[ov_core, ov_slot] = scf[is_ov]

    rhs = np.zeros((N_CORES, CPC_PAD), dtype=np.float32)
    am = np.zeros((N_CORES, CPC_PAD), dtype=np.float32)
    bm = np.zeros((N_CORES, CPC_PAD), dtype=np.float32)
    r = np.asarray(constr_rhs).reshape(N_CORES, CPC)
    s = np.asarray(constr_sense).reshape(N_CORES, CPC)
    rhs[:, :CPC] = r
    am[:, :CPC] = ((s == 1) | (s == 3)).astype(np.float32)
    bm[:, :CPC] = ((s == 2) | (s == 3)).astype(np.float32)

    ovg = OVR // P
    core_inputs = []
    for k in range(N_CORES):
        core_inputs.append({
            "ps": ps[k].reshape(P, GPP * K),
            "cs": cs[k].reshape(P, GPP * K),
            "ovp": ovp[k].reshape(P, ovg * K),
            "ovc": ovc[k].reshape(P, ovg * K),
            "ovd": ov_dest[k].reshape(P, ovg),
            "rhs": rhs[k].reshape(P, GPP),
            "am": am[k].reshape(P, GPP),
            "bm": bm[k].reshape(P, GPP),
        })
    return core_inputs, OVR


def _build_nc(OVR, reps=1):
    ovg = OVR // P
    nc = bacc.Bacc("TRN2", target_bir_lowering=False, debug=False,
                   num_devices=N_CORES)
    ps = nc.dram_tensor("ps", [P, GPP * K], F32, kind="ExternalInput").ap()
    cs = nc.dram_tensor("cs", [P, GPP * K], F32, kind="ExternalInput").ap()
    ovp = nc.dram_tensor("ovp", [P, ovg * K], F32, kind="ExternalInput").ap()
    ovc = nc.dram_tensor("ovc", [P, ovg * K], F32, kind="ExternalInput").ap()
    ovd = nc.dram_tensor("ovd", [P, ovg], mybir.dt.int32, kind="ExternalInput").ap()
    rhs = nc.dram_tensor("rhs", [P, GPP], F32, kind="ExternalInput").ap()
    am = nc.dram_tensor("am", [P, GPP], F32, kind="ExternalInput").ap()
    bm = nc.dram_tensor("bm", [P, GPP], F32, kind="ExternalInput").ap()
    part = nc.dram_tensor("part", [1, 1], F32, kind="ExternalOutput").ap()
    axd = nc.dram_tensor("axd", [CPC_PAD + 1, 1], F32).ap()

    CH = 64
    chunks = [(i, min(CH, GPP - i)) for i in range(0, GPP, CH)]

    with tile.TileContext(nc) as tc, ExitStack() as ctx:
        io = ctx.enter_context(tc.tile_pool(name="io", bufs=3))
        work = ctx.enter_context(tc.tile_pool(name="work", bufs=3))
        tail = ctx.enter_context(tc.tile_pool(name="tail", bufs=1))
        axp = ctx.enter_context(tc.tile_pool(name="axp", bufs=1))
        psum = ctx.enter_context(tc.tile_pool(name="psum", bufs=1, space="PSUM"))

        ones = axp.tile([P, 1], F32)
        nc.vector.memset(ones[:], 1.0)

        for _ in range(reps):
            ax_sb = axp.tile([P, GPP], F32, tag="ax")
            for g0, gn in chunks:
                pt = io.tile([P, CH * K], F32, tag="pt")
                ct = io.tile([P, CH * K], F32, tag="ct")
                nc.sync.dma_start(pt[:, :gn * K], ps[:, g0 * K:(g0 + gn) * K])
                nc.sync.dma_start(ct[:, :gn * K], cs[:, g0 * K:(g0 + gn) * K])
                st = work.tile([P, CH * K], F32, tag="st")
                nc.scalar.activation(st[:, :gn * K], pt[:, :gn * K], AF.Sigmoid)
                nc.vector.tensor_mul(st[:, :gn * K], st[:, :gn * K], ct[:, :gn * K])
                nc.vector.tensor_reduce(
                    ax_sb[:, g0:g0 + gn],
                    st[:, :gn * K].rearrange("p (g r) -> p g r", r=K),
                    axis=mybir.AxisListType.X, op=mybir.AluOpType.add)

            op_t = tail.tile([P, ovg * K], F32, tag="ovp")
            oc_t = tail.tile([P, ovg * K], F32, tag="ovc")
            od_t = tail.tile([P, ovg], mybir.dt.int32, tag="ovd")
            nc.sync.dma_start(op_t[:], ovp[:])
            nc.sync.dma_start(oc_t[:], ovc[:])
            nc.sync.dma_start(od_t[:], ovd[:])
            nc.scalar.activation(op_t[:], op_t[:], AF.Sigmoid)
            nc.vector.tensor_mul(op_t[:], op_t[:], oc_t[:])
            ovsum = tail.tile([P, ovg], F32, tag="ovsum")
            nc.vector.tensor_reduce(
                ovsum[:], op_t[:].rearrange("p (g r) -> p g r", r=K),
                axis=mybir.AxisListType.X, op=mybir.AluOpType.add)

            axd_main = axd[:CPC_PAD, 0].rearrange("(p g) -> p g", p=P)
            nc.sync.dma_start(axd_main, ax_sb[:])
            for j in range(ovg):
                nc.gpsimd.indirect_dma_start(
                    out=axd[:],
                    out_offset=bass.IndirectOffsetOnAxis(ap=od_t[:, j:j + 1], axis=0),
                    in_=ovsum[:, j:j + 1],
                    in_offset=None,
                    compute_op=mybir.AluOpType.add)

            axf = tail.tile([P, GPP], F32, tag="axf")
            nc.sync.dma_start(axf[:], axd_main)

            rhs_t = tail.tile([P, GPP], F32, tag="rhs")
            am_t = tail.tile([P, GPP], F32, tag="am")
            bm_t = tail.tile([P, GPP], F32, tag="bm")
            nc.sync.dma_start(rhs_t[:], rhs[:])
            nc.sync.dma_start(am_t[:], am[:])
            nc.sync.dma_start(bm_t[:], bm[:])

            d_t = tail.tile([P, GPP], F32, tag="d")
            nc.vector.tensor_tensor(out=d_t[:], in0=axf[:], in1=rhs_t[:],
                                    op=mybir.AluOpType.subtract)
            rp = tail.tile([P, GPP], F32, tag="rp")
            nc.scalar.activation(rp[:], d_t[:], AF.Relu)
            rn = tail.tile([P, GPP], F32, tag="rn")
            nc.scalar.activation(rn[:], d_t[:], AF.Relu, scale=-1.0)
            nc.vector.tensor_mul(rp[:], rp[:], am_t[:])
            nc.vector.tensor_mul(rn[:], rn[:], bm_t[:])
            nc.vector.tensor_add(rp[:], rp[:], rn[:])
            vs = tail.tile([P, 1], F32, tag="vs")
            nc.vector.tensor_reduce(vs[:], rp[:], axis=mybir.AxisListType.X,
                                    op=mybir.AluOpType.add)
            ptile = psum.tile([1, 1], F32, tag="acc")
            nc.tensor.matmul(ptile[:], lhsT=ones[:], rhs=vs[:], start=True, stop=True)
            res = tail.tile([1, 1], F32, tag="res")
            nc.vector.tensor_copy(res[:], ptile[:])
            nc.sync.dma_start(part[:], res[:])

    nc.compile()
    return nc


def kernel(pred, constr_idx, var_idx, coeff, constr_rhs, constr_sense,
           n_vars=N_VARS, n_constrs=N_CONSTRS, **_unused):
    pred = np.asarray(pred)
    constr_idx = np.asarray(constr_idx)
    var_idx = np.asarray(var_idx)
    coeff = np.asarray(coeff)
    constr_rhs = np.asarray(constr_rhs)
    constr_sense = np.asarray(constr_sense)
    assert constr_idx.shape[0] == NNZ and pred.shape[0] == N_VARS
    assert constr_rhs.shape[0] == N_CONSTRS

    core_inputs, OVR = _prep(pred, constr_idx, var_idx, coeff,
                             constr_rhs, constr_sense)
    nc = _build_nc(OVR)
    res = run_bass_kernel_spmd(nc, core_inputs, list(range(N_CORES)))
    partials = np.array([res.results[i]["part"][0, 0] for i in range(N_CORES)],
                        dtype=np.float32)
    return np.float32(partials.sum(dtype=np.float32) / np.float32(N_CONSTRS))
